# revision 39
# baseline (speedup 1.0000x reference)
"""AttnBlock (GroupNorm -> single-head attention over 64x64 pixels -> out conv
-> residual) on 8 Trainium2 NeuronCores.

Sharding: batch (B=4) x 2-way query-pixel split => 8 cores, no collectives.
Each core gets its batch element's full x in core-local pixel order
[q-half | other-half] as fp16; its 2048 query columns are always cols 0..2047.

Fast path (used when gn_bias == 0 and bq == 0; a generic baseline build covers
the rest).  x stays RAW on chip; the GroupNorm shift/scale are folded into
weights and per-channel biases (M = Wq^T Wk and W2 = Wo Wv are built on the
HOST in fp32 — the q and out convs are eliminated entirely):
  h      = a*(x - mean),  a = gn_weight * rstd
  mq     = a o (M^T (a*(xq-mean)))   conv over the 2048 query pixels only
  scores = mq^T x_raw                (the k-side -mean/a shift is a per-query
                                      constant -> cancels in softmax: the k
                                      conv runs on RAW x with NO bias at all)
  out    = x + sum_k w[q,k] v'[:,k] + (Wo bv + bo - (W2 a) mean),
           v' = (W2 a) x
           (softmax weights sum to 1, so constant v-shifts fold into the bias;
            the residual is added on the PE as an identity matmul into PSUM)
Softmax weights are transposed into the AV layout eagerly per query chunk:
all through the DMA xbar transpose (~150 GB/s, otherwise-idle engines),
issued per-quarter right after its normalize; wT is double-buffered and only
due a full scores+AV cycle later, so the transposes never block the PE.

fp16 data path: halves DMA + DVE time vs fp32, keeps LDWEIGHTS 2-byte, and
runs matmuls at the full 1 cycle/row PE rate.
"""

import numpy as np

B, C, H, W = 4, 512, 64, 64
N = H * W              # 4096 pixels
NQ = N // 2            # 2048 query pixels per core
NUM_GROUPS = 32
GSIZE = C // NUM_GROUPS  # 16 channels per group
EPS = 1e-6
SCALE = float(C) ** 0.5  # reference multiplies scores by sqrt(C)

P = 128                # partitions
CC = C // P            # 4 channel chunks
QCH = NQ // P          # 16 query chunks per core
NKQ = 1024             # k-columns per score quarter
NQW = N // NKQ         # 4 quarters per query chunk

_CACHE = {}


def _build_fast():
    from contextlib import ExitStack

    import concourse.bacc as bacc
    import concourse.tile as tile
    from concourse import mybir
    from concourse.masks import make_identity

    dt = mybir.dt

    nc = bacc.Bacc()
    x_ext = nc.declare_dram_parameter("x", [C, N], dt.float16, isOutput=False)
    mT_ext = nc.declare_dram_parameter("mT", [C, C], dt.float16, isOutput=False)
    w2aT_ext = nc.declare_dram_parameter("w2aT", [C, C], dt.float16, isOutput=False)
    # gsel | gn_ab | bias2 packed per channel: [C, 35] f32
    spack_ext = nc.declare_dram_parameter("spack", [C, NUM_GROUPS + 3], dt.float32,
                                          isOutput=False)
    esel_ext = nc.declare_dram_parameter("esel", [NUM_GROUPS, C], dt.float32, isOutput=False)
    out_ext = nc.declare_dram_parameter("out", [C, NQ], dt.float32, isOutput=True)

    with tile.TileContext(nc) as tc:
        top = ExitStack()
        const = top.enter_context(tc.tile_pool(name="const", bufs=1, side="left"))
        x_pool = top.enter_context(tc.tile_pool(name="x_pool", bufs=1, side="left"))
        mq_pool = top.enter_context(tc.tile_pool(name="mq_pool", bufs=1, side="left"))
        vT_pool = top.enter_context(tc.tile_pool(name="vT_pool", bufs=1, side="left"))
        w2_pool = top.enter_context(tc.tile_pool(name="w2_pool", bufs=1, side="left"))

        ob_sb = const.tile([P, CC], dt.float32)         # Wo bv + bo - (W2a) mean
        ident = const.tile([P, P], dt.float16)
        make_identity(nc, ident[:])

        # ax (a*(xq-mean), fp16) lives on the right side: dead once the mq
        # conv has consumed it, freeing its 16KB before the attention phase.
        ax_stack = ExitStack()
        ax_pool = ax_stack.enter_context(tc.tile_pool(name="ax_pool", bufs=1, side="right"))

        # GN scratch + build psum (right side, freed before the conv phase)
        gn_stack = ExitStack()
        affine = gn_stack.enter_context(tc.tile_pool(name="affine", bufs=1, side="right"))
        stat_pool = gn_stack.enter_context(tc.tile_pool(name="stat_pool", bufs=1, side="right"))
        small = gn_stack.enter_context(tc.tile_pool(name="small", bufs=1, side="right"))
        ps_small = gn_stack.enter_context(
            tc.tile_pool(name="ps_small", bufs=1, space="PSUM", side="right"))
        ps_build = gn_stack.enter_context(
            tc.tile_pool(name="ps_build", bufs=2, space="PSUM", side="right"))

        eps_sb = small.tile([NUM_GROUPS, 1], dt.float32)
        nc.vector.memset(eps_sb[:], EPS)

        # x chunk 0 first (GroupNorm stats are the serial front chain), then
        # the host-packed small tensors (ONE partition-contiguous DMA — the
        # strided per-tensor layouts cost ~1500 tiny descriptors that stall
        # the queue ahead of x), M (host-built Wq^T Wk), the rest of x, then
        # W2 (host-built (Wo Wv)^T).
        x_t = [x_pool.tile([P, N // 512, 512], dt.float16, name=f"x_{cc}",
                           tag=f"x_{cc}") for cc in range(CC)]

        def dma_x(cc, split=1):
            nb = (N // 512) // split
            for h in range(split):
                nc.sync.dma_start(
                    out=x_t[cc][:, h * nb:(h + 1) * nb, :],
                    in_=x_ext[cc * P:(cc + 1) * P, h * nb * 512:(h + 1) * nb * 512]
                        .rearrange("p (b k) -> p b k", k=512))

        dma_x(0, split=2)
        spack_sb = small.tile([P, CC, NUM_GROUPS + 3], dt.float32)
        nc.sync.dma_start(out=spack_sb[:],
                          in_=spack_ext.rearrange("(c p) k -> p c k", p=P))
        gsel_sb = spack_sb[:, :, 0:NUM_GROUPS]
        gn_ab_sb = spack_sb[:, :, NUM_GROUPS:NUM_GROUPS + 2]
        bias2_sb = spack_sb[:, :, NUM_GROUPS + 2:NUM_GROUPS + 3]
        esel_sb = small.tile([NUM_GROUPS, C], dt.float32)
        nc.sync.dma_start(out=esel_sb[:], in_=esel_ext[:])
        mT_sb = w2_pool.tile([P, CC, C], dt.float16)
        nc.sync.dma_start(out=mT_sb[:], in_=mT_ext.rearrange("(c p) k -> p c k", p=P))
        for cc in range(1, CC):
            dma_x(cc, split=2)
        W2aT_sb = w2_pool.tile([P, CC, C], dt.float16)
        nc.sync.dma_start(out=W2aT_sb[:], in_=w2aT_ext.rearrange("(c p) k -> p c k", p=P))

        # 5 PSUM banks stage the first five mq-conv output tiles: their
        # ic-pass matmuls run as each chunk's stats/ax complete, filling the
        # PE during the stats-gated front phase.  (qt, jc) staged tiles:
        mq_staged = [(0, 0), (0, 1), (0, 2), (0, 3), (1, 0)]
        gs_stack = ExitStack()
        ps_gs = gs_stack.enter_context(
            tc.tile_pool(name="ps_gs", bufs=1, space="PSUM", side="right"))
        gs_t = [ps_gs.tile([P, 512], dt.float32, name=f"gs_{t}", tag=f"gs_{t}")
                for t in range(len(mq_staged))]
        ps_warm = gs_stack.enter_context(
            tc.tile_pool(name="ps_warm", bufs=1, space="PSUM", side="right"))
        warm_t = ps_warm.tile([P, 512], dt.float32, tag="warm", bufs=1)
        dum_sb = const.tile([P, 512], dt.float16)
        nc.gpsimd.memset(dum_sb[:], 0.0)

        def warm_fill(n, lhsT=None):
            """512-row matmuls that keep the PE busy (and the HAM clock-gate
            warm) while real work waits on DMA / GroupNorm stats.  lhsT picks
            the dependency that pins the batch to the right phase — the
            scheduler orders by dependencies, not program order."""
            for _ in range(n):
                nc.tensor.matmul(warm_t[:, 0:64], ident[:] if lhsT is None else lhsT,
                                 dum_sb[:, 0:64], start=True, stop=True)

        # staggered warm bursts pinned to successive front DMAs: keeps the
        # HAM activity window from re-throttling across the stats-gated wait
        warm_fill(12)
        warm_fill(8, lhsT=x_t[0][:, 0, 0:P])
        warm_fill(8, lhsT=x_t[0][:, 4, 0:P])
        warm_fill(8, lhsT=mT_sb[:, 0, 0:P])
        warm_fill(8, lhsT=x_t[1][:, 4, 0:P])
        warm_fill(8, lhsT=x_t[2][:, 4, 0:P])
        warm_fill(8, lhsT=mT_sb[:, 3, 0:P])

        # ---------------- GroupNorm stats -> a = gnw*rstd, -mean ------------
        # x stays RAW in SBUF; the GN shift is folded into per-channel biases:
        #   ax      = a*(xq - mean)            (fused scalar activation)
        #   g bias  = -(M a) mean, applied on the g copies (per-partition)
        #   v' bias = -(W2 a) mean, constant after sum_k w = 1 -> into ob
        #   out     = x + attn_raw + (W2a)(-mean) + Wo bv + bo
        ax_t = [None] * CC
        ab_t = [None] * CC
        g_t = [None] * CC
        nm16 = small.tile([P, CC, 1], dt.float16)

        def stats_a(cc):
            """bn_stats burst + group-sum for chunk cc — DVE ops here have
            only local deps, so the strict-FIFO DVE queue never stalls."""
            mv = stat_pool.tile([P, 2], dt.float32, name=f"mv_{cc}", tag="mv", bufs=2)
            stats = stat_pool.tile([P, 8, 6], dt.float32, name=f"st_{cc}",
                                   tag="st", bufs=2)
            for j in range(8):
                nc.vector.bn_stats(out=stats[:, j, :], in_=x_t[cc][:, j, :])
            nc.vector.bn_aggr(out=mv[:], in_=stats[:])
            # mv[:,1] := var + mean^2 (per-channel second moment)
            sq = stat_pool.tile([P, 1], dt.float32, name=f"sq_{cc}", tag="sq", bufs=2)
            nc.vector.tensor_mul(sq[:], mv[:, 0:1], mv[:, 0:1])
            nc.vector.tensor_add(mv[:, 1:2], mv[:, 1:2], sq[:])

            gps = ps_small.tile([P, 2], dt.float32, tag="statps", bufs=1)
            nc.tensor.matmul(gps[0:NUM_GROUPS, :], gsel_sb[:, cc, :], mv[:],
                             start=True, stop=True)
            g_sb = small.tile([NUM_GROUPS, 2], dt.float32, name=f"g_{cc}", tag="g", bufs=2)
            nc.scalar.copy(g_sb[:], gps[0:NUM_GROUPS, :])
            g_t[cc] = g_sb

        def stats_b(cc):
            """group rstd + per-channel fold for chunk cc.  Emitted AFTER
            chunk cc+1's stats_a so its cross-engine round trips never block
            the next bn_stats burst in the DVE queue."""
            g_sb = g_t[cc]
            # var_g = E[x^2] - mean^2 ; rstd = exp(-0.5*ln(var+eps))
            gm2 = small.tile([NUM_GROUPS, 1], dt.float32, name=f"gm2_{cc}", tag="gm2", bufs=2)
            nc.vector.tensor_mul(gm2[:], g_sb[:, 0:1], g_sb[:, 0:1])
            grp = small.tile([NUM_GROUPS, 2], dt.float32, name=f"grp_{cc}", tag="grp", bufs=2)
            nc.vector.tensor_copy(grp[:, 0:1], g_sb[:, 0:1])
            varg = small.tile([NUM_GROUPS, 1], dt.float32, name=f"varg_{cc}", tag="varg", bufs=2)
            nc.vector.tensor_sub(varg[:], g_sb[:, 1:2], gm2[:])
            lng = small.tile([NUM_GROUPS, 1], dt.float32, name=f"lng_{cc}", tag="lng", bufs=2)
            nc.scalar.activation(lng[:], varg[:], mybir.ActivationFunctionType.Ln,
                                 bias=eps_sb[:], scale=1.0)
            nc.scalar.activation(grp[:, 1:2], lng[:], mybir.ActivationFunctionType.Exp,
                                 bias=0.0, scale=-0.5)

            # broadcast (mean, rstd) to this chunk's channels
            pcs = ps_small.tile([P, 2], dt.float32, tag="statps", bufs=1)
            nc.tensor.matmul(pcs[:], esel_sb[:, cc * P:(cc + 1) * P], grp[:],
                             start=True, stop=True)
            # a = gnw * rstd ; nm = -mean.  The last chunk's chain is the
            # exposed critical path: read the broadcast PSUM directly on DVE
            # and skip the ACT staging copy.
            if cc == CC - 1:
                pc = pcs
            else:
                pc = small.tile([P, 2], dt.float32, name=f"pc_{cc}", tag="pc", bufs=2)
                nc.scalar.copy(pc[:], pcs[:])
            ab = affine.tile([P, 2], dt.float32, name=f"ab_{cc}", tag=f"ab_{cc}")
            nc.vector.tensor_mul(ab[:, 0:1], gn_ab_sb[:, cc, 0:1], pc[:, 1:2])
            ab_t[cc] = ab
            nc.vector.tensor_scalar_mul(nm16[:, cc, :], pc[:, 0:1], -1.0)
            # ax = a*x - a*mean (one scalar activation)
            nm2 = small.tile([P, 1], dt.float32, name=f"nm2_{cc}", tag="nm2", bufs=2)
            nc.vector.tensor_mul(nm2[:], pc[:, 0:1], ab[:, 0:1])
            nc.vector.tensor_scalar_mul(nm2[:], nm2[:], -1.0)
            axt = ax_pool.tile([P, NQ // 512, 512], dt.float16, name=f"ax_{cc}",
                               tag=f"ax_{cc}")
            nc.scalar.activation(out=axt[:], in_=x_t[cc][:, 0:NQ // 512, :],
                                 func=mybir.ActivationFunctionType.Identity,
                                 bias=nm2[:], scale=ab[:, 0:1])
            ax_t[cc] = axt
            # staged mq-conv pass for this chunk (contracts i = chunk cc)
            for t, (qt, jc) in enumerate(mq_staged):
                nc.tensor.matmul(gs_t[t][:], mT_sb[:, cc, jc * P:(jc + 1) * P],
                                 axt[:, qt, :],
                                 start=(cc == 0), stop=(cc == CC - 1))
            if cc < CC - 1:
                warm_fill(16, lhsT=mT_sb[:, cc, 0:P])

        stats_a(0)
        stats_a(1)
        stats_b(0)
        stats_a(2)
        stats_b(1)
        stats_a(3)
        stats_b(2)
        stats_b(3)

        # a-fold of W2; v' bias fold (tiny matmuls against -mean).  The k-side
        # score shift -mq^T(a*mean) is a per-query constant -> cancels in
        # softmax, so no g-bias is needed at all.
        for cc in range(CC):
            nc.vector.tensor_scalar_mul(W2aT_sb[:, cc, :], W2aT_sb[:, cc, :],
                                        ab_t[cc][:, 0:1])
        for oc in range(CC):
            ps = ps_build.tile([P, C], dt.float32, tag="build", bufs=1)
            for cc in range(CC):
                nc.tensor.matmul(ps[:, 0:1], W2aT_sb[:, cc, oc * P:(oc + 1) * P],
                                 nm16[:, cc, :], start=(cc == 0), stop=(cc == CC - 1))
            nc.vector.tensor_add(ob_sb[:, oc:oc + 1], bias2_sb[:, oc, 0:1],
                                 ps[:, 0:1])

        # staged mq tiles -> SBUF (a_j fold rides the copy), free their banks
        mq_sb = mq_pool.tile([P, CC, NQ], dt.float16)
        def mq_copy(qt, jc, src, on_vector):
            dst = mq_sb[:, jc, qt * 512:(qt + 1) * 512]
            if on_vector:
                nc.vector.tensor_scalar_mul(dst, src, ab_t[jc][:, 0:1])
            else:
                nc.scalar.activation(out=dst, in_=src,
                                     func=mybir.ActivationFunctionType.Identity,
                                     bias=0.0, scale=ab_t[jc][:, 0:1])

        for t, (qt, jc) in enumerate(mq_staged):
            mq_copy(qt, jc, gs_t[t][:], t % 2 == 0)
        gs_stack.close()
        gn_stack.close()

        # ---------------- mq conv (rest) and v' conv -------------------------
        conv_ps_stack = ExitStack()
        ps_conv = conv_ps_stack.enter_context(
            tc.tile_pool(name="ps_conv", bufs=4, space="PSUM", side="right"))

        for qt in range(NQ // 512):
            for jc in range(CC):
                if (qt, jc) in mq_staged:
                    continue
                ps = ps_conv.tile([P, 512], dt.float32, tag="conv", bufs=4)
                for cc in range(CC):
                    nc.tensor.matmul(ps[:], mT_sb[:, cc, jc * P:(jc + 1) * P],
                                     ax_t[cc][:, qt, :],
                                     start=(cc == 0), stop=(cc == CC - 1))
                mq_copy(qt, jc, ps[:], jc % 2 == 0)

        conv_ps_stack.close()
        ax_stack.close()

        # ---------------- attention ------------------------------------------
        at_stack = ExitStack()
        at = at_stack.enter_context(tc.tile_pool(name="at", bufs=2, side="left"))
        wT_pool = at_stack.enter_context(tc.tile_pool(name="wT_pool", bufs=1, side="left"))
        out_pool = at_stack.enter_context(tc.tile_pool(name="out_pool", bufs=2, side="left"))
        ps_sc = at_stack.enter_context(
            tc.tile_pool(name="ps_sc", bufs=2, space="PSUM", side="left"))
        ps_at = at_stack.enter_context(
            tc.tile_pool(name="ps_at", bufs=4, space="PSUM", side="left"))

        # v'T[pix, o] = sum_c xm[c, pix] W2a[c, o] — emitted in slices
        # interleaved with the first group's scores so its matmuls fill the
        # softmax-latency bubbles of the not-yet-pipelined PE stream.  Uses
        # the (still idle) AV psum ring.
        vT_sb = vT_pool.tile([P, N // P, C], dt.float16)

        def v_conv_slice(i0, i1):
            for pc_i in range(i0, i1):
                ps = ps_at.tile([P, C], dt.float32, tag="at", bufs=4)
                for cc in range(CC):
                    nc.tensor.matmul(ps[:],
                                     x_t[cc][:, pc_i // 4, (pc_i % 4) * P:(pc_i % 4 + 1) * P],
                                     W2aT_sb[:, cc, :], start=(cc == 0), stop=(cc == CC - 1))
                if pc_i % 2 == 0:
                    nc.vector.tensor_copy(vT_sb[:, pc_i, :], ps[:])
                else:
                    nc.scalar.copy(vT_sb[:, pc_i, :], ps[:])

        def scores_softmax(qi, wT_dst):
            """scores + online softmax for query chunk qi -> normalized e,
            eagerly transposed into wT_dst's [k-part, kc, q] layout.  The DMA
            xbar (out[p, j, q] = in[q, j*128+p] — measured ~150 GB/s) takes
            quarters 0-1; the PE identity-transpose takes quarters 2-3.  Eager
            issue spreads the transposes over this group's own score window,
            and the double-buffered wT means no WAR stall on the prior AV."""
            qi4 = qi % 4
            e_q = at.tile([P, NQW, NKQ], dt.float16, tag=f"e{qi4}", bufs=1,
                          name=f"e_{qi}")
            mx = at.tile([P, NQW], dt.float32, tag="mx")
            sq = at.tile([P, NQW], dt.float32, tag="sq")
            bias_t = at.tile([P, NQW], dt.float32, tag="bias")
            for w in range(NQW):
                ps = ps_sc.tile([P, NKQ], dt.float32, tag="sc", bufs=2)
                for half in range(2):
                    for cc in range(CC):
                        nc.tensor.matmul(ps[:, half * 512:(half + 1) * 512],
                                         mq_sb[:, cc, qi * P:(qi + 1) * P],
                                         x_t[cc][:, w * 2 + half, :],
                                         start=(cc == 0), stop=(cc == CC - 1))
                nc.vector.reduce_max(out=mx[:, w:w + 1], in_=ps[:],
                                     axis=mybir.AxisListType.X)
                nc.vector.tensor_scalar_mul(bias_t[:, w:w + 1], mx[:, w:w + 1],
                                            -SCALE)
                nc.scalar.activation(
                    out=e_q[:, w, :], in_=ps[:],
                    func=mybir.ActivationFunctionType.Exp,
                    bias=bias_t[:, w:w + 1], scale=SCALE,
                    accum_out=sq[:, w:w + 1])
            # combine quarters: m = max_w mx ; alpha_w = exp(SCALE*(mx-m))/s
            m_t = at.tile([P, 1], dt.float32, tag="m")
            nc.vector.reduce_max(out=m_t[:], in_=mx[:], axis=mybir.AxisListType.X)
            mb = at.tile([P, 1], dt.float32, tag="mb")
            nc.vector.tensor_scalar_mul(mb[:], m_t[:], -SCALE)
            beta = at.tile([P, NQW], dt.float32, tag="beta")
            nc.scalar.activation(out=beta[:], in_=mx[:],
                                 func=mybir.ActivationFunctionType.Exp,
                                 bias=mb[:], scale=SCALE)
            sb_t = at.tile([P, NQW], dt.float32, tag="sbt")
            nc.vector.tensor_mul(sb_t[:], sq[:], beta[:])
            s_t = at.tile([P, 1], dt.float32, tag="s")
            nc.vector.reduce_sum(out=s_t[:], in_=sb_t[:], axis=mybir.AxisListType.X)
            rs = at.tile([P, 1], dt.float32, tag="rs")
            nc.vector.reciprocal(rs[:], s_t[:])
            alpha = at.tile([P, NQW], dt.float32, tag="alpha")
            nc.vector.tensor_scalar_mul(alpha[:], beta[:], rs[:])
            # eager transpose: e[q-part, k] -> wT[k-part, kc, q], all through
            # the DMA xbar.  Issued per-quarter right after its normalize, and
            # due only at AV(qg) — a full scores+AV cycle (~55us) later — so
            # the ~150 GB/s xbar (73 GB/s steady-state demand) never gates.
            for w in range(NQW):
                if w % 2 == 0:
                    nc.vector.tensor_scalar_mul(e_q[:, w, :], e_q[:, w, :],
                                                alpha[:, w:w + 1])
                else:
                    nc.scalar.activation(out=e_q[:, w, :], in_=e_q[:, w, :],
                                         func=mybir.ActivationFunctionType.Identity,
                                         bias=0.0, scale=alpha[:, w:w + 1])
                nc.sync.dma_start_transpose(
                    out=wT_dst[:, w * 8:(w + 1) * 8, qi4 * P:(qi4 + 1) * P],
                    in_=e_q[:, w, :])
            return e_q

        def make_wT(qg):
            return wT_pool.tile([P, N // P, 512], dt.float16, tag=f"wT{qg % 2}",
                                name=f"wT_{qg}")

        wT_cur = make_wT(0)
        eq_t = []
        for qi in range(4):
            eq_t.append(scores_softmax(qi, wT_cur))
            v_conv_slice(qi * 8, (qi + 1) * 8)

        for qg in range(QCH // 4):  # groups of 4 query chunks (512 queries)
            wT_this = wT_cur
            # --- next group's scores+softmax overlap this group's attn ---
            if qg < QCH // 4 - 1:
                wT_cur = make_wT(qg + 1)
                eq_t = [scores_softmax((qg + 1) * 4 + k, wT_cur) for k in range(4)]

            # --- attn = v' @ weights^T for this 512-query group ---
            # kc-outer in oc-pairs: each wT[kc] slice is fully consumed early.
            # The very last pair runs oc-sequential instead, so the first
            # oc's output chain (copy + residual + DMA) hides under the
            # second oc's matmuls and only one chain remains in the tail.
            for oc0 in (0, 2):
                def emit_out(oc, ps):
                    # out = attn + xm + (Wo bv + bo + mean); the residual x
                    # rides a GpSimd add (off the PE stream)
                    o_sb = out_pool.tile([P, 512], dt.float32, tag="o", bufs=2)
                    nc.scalar.activation(out=o_sb[:], in_=ps[:],
                                         func=mybir.ActivationFunctionType.Identity,
                                         bias=ob_sb[:, oc:oc + 1], scale=1.0)
                    # final group: DVE add (GpSimd dispatch latency would sit
                    # squarely in the kernel tail)
                    if qg == QCH // 4 - 1:
                        nc.vector.tensor_add(o_sb[:], o_sb[:], x_t[oc][:, qg, :])
                    else:
                        nc.gpsimd.tensor_add(o_sb[:], o_sb[:], x_t[oc][:, qg, :])
                    nc.sync.dma_start(
                        out=out_ext[oc * P:(oc + 1) * P, qg * 512:(qg + 1) * 512],
                        in_=o_sb[:])

                if qg == QCH // 4 - 1 and oc0 == 2:
                    for j in range(2):
                        oc = oc0 + j
                        psj = ps_at.tile([P, 512], dt.float32, tag="at", bufs=4,
                                         name=f"at_ps_last_{j}")
                        for kc in range(N // P):
                            nc.tensor.matmul(psj[:],
                                             vT_sb[:, kc, oc * P:(oc + 1) * P],
                                             wT_this[:, kc, :],
                                             start=(kc == 0),
                                             stop=(kc == N // P - 1))
                        emit_out(oc, psj)
                    continue
                ps_pair = [ps_at.tile([P, 512], dt.float32, tag="at", bufs=4,
                                      name=f"at_ps_{oc0}_{j}")
                           for j in range(2)]
                for kc in range(N // P):
                    for j in range(2):
                        oc = oc0 + j
                        nc.tensor.matmul(ps_pair[j][:],
                                         vT_sb[:, kc, oc * P:(oc + 1) * P],
                                         wT_this[:, kc, :],
                                         start=(kc == 0), stop=(kc == N // P - 1))
                for j in range(2):
                    emit_out(oc0 + j, ps_pair[j])
        at_stack.close()
        top.close()

    # Force every activation onto the natural_log_exp_and_others table set so
    # the kernel never pays a mid-run ACT table swap (~2.7us each).
    import concourse.bacc as bacc_mod
    orig_tables = bacc_mod.get_activation_tables

    def one_set_tables(arch):
        t = dict(orig_tables(arch))
        return {name: (funcs if name == "natural_log_exp_and_others" else frozenset())
                for name, funcs in t.items()}

    bacc_mod.get_activation_tables = one_set_tables
    try:
        nc.compile()
    finally:
        bacc_mod.get_activation_tables = orig_tables
    return nc


def _get_nc_fast():
    if "fast" not in _CACHE:
        _CACHE["fast"] = _build_fast()
    return _CACHE["fast"]


def _in_maps_fast(x, gn_weight, gn_bias, wq, bq, wk, bk, wv, bv, wo, bo):
    x = np.asarray(x, dtype=np.float32)
    f32 = lambda a: np.ascontiguousarray(np.asarray(a, dtype=np.float32))
    f16 = lambda a: np.ascontiguousarray(np.asarray(a, dtype=np.float16))

    wq32 = np.asarray(wq, dtype=np.float32)
    wk32 = np.asarray(wk, dtype=np.float32)
    wv32 = np.asarray(wv, dtype=np.float32)
    wo32 = np.asarray(wo, dtype=np.float32)
    # M[i, j] = sum_o Wq[o, i] Wk[o, j]; W2aT[c, o] = (Wo Wv)^T[c, o]
    mT16 = f16(wq32.T @ wk32)
    w2aT16 = f16((wo32 @ wv32).T)
    bias2 = f32((wo32 @ np.asarray(bv, np.float32)
                 + np.asarray(bo, np.float32)).reshape(C, 1))
    gn_ab = f32(np.stack([gn_weight, gn_bias], axis=1))     # [C, 2]

    gsel = np.zeros((C, NUM_GROUPS), dtype=np.float32)
    gsel[np.arange(C), np.arange(C) // GSIZE] = 1.0 / GSIZE
    esel = np.zeros((NUM_GROUPS, C), dtype=np.float32)
    esel[np.arange(C) // GSIZE, np.arange(C)] = 1.0
    # gsel | gn_ab | bias2 packed per channel -> one partition-contiguous DMA
    spack = f32(np.concatenate([gsel, gn_ab, bias2], axis=1))  # [C, 35]

    in_maps = []
    for core in range(8):
        b, half = core // 2, core % 2
        xb = x[b].reshape(C, N)
        xc = np.concatenate(
            [xb[:, half * NQ:(half + 1) * NQ], xb[:, (1 - half) * NQ:(2 - half) * NQ]],
            axis=1)
        in_maps.append({
            "x": f16(xc),
            "mT": mT16, "w2aT": w2aT16,
            "spack": spack, "esel": esel,
        })
    return in_maps


def _gather(res):
    out = np.empty((B, C, N), dtype=np.float32)
    for core in range(8):
        b, half = core // 2, core % 2
        out[b, :, half * NQ:(half + 1) * NQ] = res.results[core]["out"]
    return out.reshape(B, C, H, W)


GEN_QK_MODE = "fp32r"


def _build_generic(qk_mode, repeats=1):
    from contextlib import ExitStack

    import concourse.bacc as bacc
    import concourse.tile as tile
    from concourse import mybir
    from concourse.masks import make_identity

    dt = mybir.dt
    qk_dt = dt.float32r if qk_mode == "fp32r" else dt.float32

    nc = bacc.Bacc()
    xq_ext = nc.declare_dram_parameter("xq", [C, NQ], dt.float32, isOutput=False)
    xo_ext = nc.declare_dram_parameter("xo", [C, NQ], dt.float32, isOutput=False)
    wqT_ext = nc.declare_dram_parameter("wqT", [C, C], dt.float32, isOutput=False)
    wkT_ext = nc.declare_dram_parameter("wkT", [C, C], dt.float32, isOutput=False)
    wvT_ext = nc.declare_dram_parameter("wvT", [C, C], dt.float32, isOutput=False)
    woT_ext = nc.declare_dram_parameter("woT", [C, C], dt.float32, isOutput=False)
    biases_ext = nc.declare_dram_parameter("biases", [C, 4], dt.float32, isOutput=False)
    gn_ab_ext = nc.declare_dram_parameter("gn_ab", [C, 2], dt.float32, isOutput=False)
    gsel_ext = nc.declare_dram_parameter("gsel", [C, NUM_GROUPS], dt.float32, isOutput=False)
    esel_ext = nc.declare_dram_parameter("esel", [NUM_GROUPS, C], dt.float32, isOutput=False)
    out_ext = nc.declare_dram_parameter("out", [C, NQ], dt.float32, isOutput=True)

    with tile.TileContext(nc) as tc:
        # LEFT side: long-lived pools (whole kernel / attention phase).
        # RIGHT side: phase-scoped pools (GN scratch, conv weights, h).
        for _rep in range(repeats):
            top = ExitStack()
            const = top.enter_context(tc.tile_pool(name="const", bufs=1, side="left"))
            biases_sb = const.tile([P, CC, 4], dt.float32)  # [:, :, 0..3] = bq, bk, bv, bo
            nc.sync.dma_start(out=biases_sb[:], in_=biases_ext.rearrange("(c p) k -> p c k", p=P))
            k_pool = top.enter_context(tc.tile_pool(name="k_pool", bufs=1, side="left"))
            vT_pool = top.enter_context(tc.tile_pool(name="vT_pool", bufs=1, side="left"))

            # ---------------- Phase 1+2: GroupNorm folded into convs ----------------
            # GroupNorm h = a*x + b is folded into the conv weights:
            #   W' = W @ diag(a),  bias' = W @ b + bias
            # so K/V/Q are computed directly from x and h never materializes.
            # Group stats are per channel-chunk (groups never span chunks), so
            # chunk cc's conv matmuls start as soon as its own stats are done.
            bx_sb = const.tile([P, CC, 3], dt.float32)  # folded conv biases q,k,v
            q_pool = top.enter_context(tc.tile_pool(name="q_pool", bufs=1, side="left"))

            hq_stack = ExitStack()
            hq_pool = hq_stack.enter_context(tc.tile_pool(name="hq_pool", bufs=1, side="right"))
            ho_stack = ExitStack()
            ho_pool = ho_stack.enter_context(tc.tile_pool(name="ho_pool", bufs=1, side="right"))
            af_stack = ExitStack()
            affine = af_stack.enter_context(tc.tile_pool(name="affine", bufs=1, side="right"))
            gn_stack = ExitStack()
            stat_pool = gn_stack.enter_context(tc.tile_pool(name="stat_pool", bufs=1, side="right"))
            small = gn_stack.enter_context(tc.tile_pool(name="small", bufs=1, side="right"))
            ps_small = gn_stack.enter_context(
                tc.tile_pool(name="ps_small", bufs=1, space="PSUM", side="right"))

            gsel_sb = small.tile([P, CC, NUM_GROUPS], dt.float32)
            nc.sync.dma_start(out=gsel_sb[:], in_=gsel_ext.rearrange("(c p) g -> p c g", p=P))
            esel_sb = small.tile([NUM_GROUPS, C], dt.float32)
            nc.sync.dma_start(out=esel_sb[:], in_=esel_ext[:])
            gn_ab_sb = small.tile([P, CC, 2], dt.float32)
            nc.sync.dma_start(out=gn_ab_sb[:], in_=gn_ab_ext.rearrange("(c p) k -> p c k", p=P))
            eps_sb = small.tile([NUM_GROUPS, 1], dt.float32)
            nc.vector.memset(eps_sb[:], EPS)

            xq_t, xo_t, ab_t = [], [], []
            for cc in range(CC):
                xqt = hq_pool.tile([P, NQ], qk_dt, name=f"hq_{cc}", tag=f"hq_{cc}")
                xot = ho_pool.tile([P, NQ], qk_dt, name=f"ho_{cc}", tag=f"ho_{cc}")
                for hcol in range(2):
                    cs = slice(hcol * NQ // 2, (hcol + 1) * NQ // 2)
                    nc.sync.dma_start(out=xqt[:, cs],
                                      in_=xq_ext[cc * P:(cc + 1) * P, cs].bitcast(qk_dt))
                    nc.sync.dma_start(out=xot[:, cs],
                                      in_=xo_ext[cc * P:(cc + 1) * P, cs].bitcast(qk_dt))
                xq_t.append(xqt)
                xo_t.append(xot)

            for cc in range(CC):
                xqf = xq_t[cc][:].bitcast(dt.float32)
                xof = xo_t[cc][:].bitcast(dt.float32)
                stats = stat_pool.tile([P, 8, 6], dt.float32, name=f"st_{cc}", tag="st", bufs=2)
                for j in range(4):
                    nc.vector.bn_stats(out=stats[:, j, :], in_=xqf[:, j * 512:(j + 1) * 512])
                for j in range(4):
                    nc.vector.bn_stats(out=stats[:, 4 + j, :], in_=xof[:, j * 512:(j + 1) * 512])
                mv = stat_pool.tile([P, 2], dt.float32, name=f"mv_{cc}", tag="mv", bufs=2)
                nc.vector.bn_aggr(out=mv[:], in_=stats[:])
                # mv[:,1] := var + mean^2  (per-channel second moment)
                sq = stat_pool.tile([P, 1], dt.float32, name=f"sq_{cc}", tag="sq", bufs=2)
                nc.vector.tensor_mul(sq[:], mv[:, 0:1], mv[:, 0:1])
                nc.vector.tensor_add(mv[:, 1:2], mv[:, 1:2], sq[:])

                # this chunk's 8 groups: [32, 2] = sum_c gsel[c,g] * mv[c,:]
                gps = ps_small.tile([NUM_GROUPS, 2], dt.float32, tag="gps", bufs=1)
                nc.tensor.matmul(gps[:], gsel_sb[:, cc, :], mv[:], start=True, stop=True)
                g_sb = small.tile([NUM_GROUPS, 2], dt.float32, name=f"g_{cc}", tag="g", bufs=2)
                nc.scalar.copy(g_sb[:], gps[:])
                # var_g = E[x^2] - mean^2 ; rstd = exp(-0.5*ln(var+eps))
                # (ln+exp live in one ACT table set; sqrt would force a table swap)
                gm2 = small.tile([NUM_GROUPS, 1], dt.float32, name=f"gm2_{cc}", tag="gm2", bufs=2)
                nc.vector.tensor_mul(gm2[:], g_sb[:, 0:1], g_sb[:, 0:1])
                grp = small.tile([NUM_GROUPS, 2], dt.float32, name=f"grp_{cc}", tag="grp", bufs=2)
                nc.vector.tensor_copy(grp[:, 0:1], g_sb[:, 0:1])
                varg = small.tile([NUM_GROUPS, 1], dt.float32, name=f"varg_{cc}", tag="varg", bufs=2)
                nc.vector.tensor_sub(varg[:], g_sb[:, 1:2], gm2[:])
                lng = small.tile([NUM_GROUPS, 1], dt.float32, name=f"lng_{cc}", tag="lng", bufs=2)
                nc.scalar.activation(lng[:], varg[:], mybir.ActivationFunctionType.Ln,
                                     bias=eps_sb[:], scale=1.0)
                nc.scalar.activation(grp[:, 1:2], lng[:], mybir.ActivationFunctionType.Exp,
                                     bias=0.0, scale=-0.5)

                # broadcast (mean, rstd) to this chunk's channels; GN affine fold:
                # a = gnw*rstd ; b = gnb - mean*a
                pcs = ps_small.tile([P, 2], dt.float32, tag="pcs", bufs=1)
                nc.tensor.matmul(pcs[:], esel_sb[:, cc * P:(cc + 1) * P], grp[:],
                                 start=True, stop=True)
                pc = small.tile([P, 2], dt.float32, name=f"pc_{cc}", tag="pc", bufs=2)
                nc.scalar.copy(pc[:], pcs[:])
                ab = affine.tile([P, 2], dt.float32, name=f"ab_{cc}", tag=f"ab_{cc}")
                nc.vector.tensor_mul(ab[:, 0:1], gn_ab_sb[:, cc, 0:1], pc[:, 1:2])
                t0 = small.tile([P, 1], dt.float32, name=f"t0_{cc}", tag="t0", bufs=2)
                nc.vector.tensor_mul(t0[:], pc[:, 0:1], ab[:, 0:1])
                nc.vector.tensor_sub(ab[:, 1:2], gn_ab_sb[:, cc, 1:2], t0[:])
                ab_t.append(ab)

            gn_stack.close()

            def x_cols(cc, col0, width):
                """x[cc][:, col0:col0+width] in the core-local order [xq | xo]."""
                if col0 < NQ:
                    return xq_t[cc][:, col0:col0 + width]
                return xo_t[cc][:, col0 - NQ:col0 - NQ + width]

            # ---------------- Phase 2: K / V / Q convs (from x directly) --------
            conv_ps_stack = ExitStack()
            ps_conv = conv_ps_stack.enter_context(
                tc.tile_pool(name="ps_conv", bufs=4, space="PSUM", side="right"))

            def fold_weight(wT_sb, bias_col):
                """Per channel chunk: bias' += W_cc^T b_cc, then scale W_cc in
                place (W'[c, o] = W[c, o] * a[c]) — each chunk gated only on its
                own group stats."""
                bacc_sb = affine.tile([P, CC], dt.float32, name=f"bacc_{bias_col}",
                                      tag=f"bacc_{bias_col}")
                for cc in range(CC):
                    for oc in range(CC):
                        bps = ps_conv.tile([P, 1], dt.float32, tag="bps", bufs=2)
                        nc.tensor.matmul(bps[:],
                                         wT_sb[:, cc, oc * P:(oc + 1) * P].bitcast(dt.float32),
                                         ab_t[cc][:, 1:2], start=True, stop=True)
                        if cc == 0:
                            nc.vector.tensor_copy(bacc_sb[:, oc:oc + 1], bps[:])
                        else:
                            nc.vector.tensor_add(bacc_sb[:, oc:oc + 1],
                                                 bacc_sb[:, oc:oc + 1], bps[:])
                    nc.vector.tensor_scalar_mul(wT_sb[:, cc, :],
                                                wT_sb[:, cc, :].bitcast(dt.float32),
                                                ab_t[cc][:, 0:1])
                for oc in range(CC):
                    nc.vector.tensor_add(bx_sb[:, oc, bias_col:bias_col + 1],
                                         bacc_sb[:, oc:oc + 1],
                                         biases_sb[:, oc, bias_col:bias_col + 1])

            wk_stack = ExitStack()
            wk_pool = wk_stack.enter_context(tc.tile_pool(name="wk_pool", bufs=1, side="right"))
            wkT_sb = wk_pool.tile([P, CC, C], qk_dt)
            for cc in range(CC):
                nc.sync.dma_start(out=wkT_sb[:, cc, :],
                                  in_=wkT_ext[cc * P:(cc + 1) * P, :].bitcast(qk_dt))
            fold_weight(wkT_sb, 1)

            k_t = [k_pool.tile([P, N], qk_dt, name=f"k_{oc}", tag=f"k_{oc}") for oc in range(CC)]
            for oc in range(CC):
                for ncol in range(N // 512):
                    ps = ps_conv.tile([P, 512], dt.float32, tag="conv", bufs=4)
                    for cc in range(CC):
                        nc.tensor.matmul(ps[:], wkT_sb[:, cc, oc * P:(oc + 1) * P],
                                         x_cols(cc, ncol * 512, 512),
                                         start=(cc == 0), stop=(cc == CC - 1))
                    if ncol % 2 == 0:
                        nc.vector.tensor_scalar(
                            out=k_t[oc][:, ncol * 512:(ncol + 1) * 512], in0=ps[:],
                            scalar1=bx_sb[:, oc, 1:2], scalar2=None,
                            op0=mybir.AluOpType.add)
                    else:
                        nc.scalar.activation(
                            out=k_t[oc][:, ncol * 512:(ncol + 1) * 512], in_=ps[:],
                            func=mybir.ActivationFunctionType.Identity,
                            bias=bx_sb[:, oc, 1:2], scale=1.0)
            wk_stack.close()

            wv_stack = ExitStack()
            wv_pool = wv_stack.enter_context(tc.tile_pool(name="wv_pool", bufs=1, side="right"))
            wvT_sb = wv_pool.tile([P, CC, C], qk_dt)
            for cc in range(CC):
                nc.sync.dma_start(out=wvT_sb[:, cc, :],
                                  in_=wvT_ext[cc * P:(cc + 1) * P, :].bitcast(qk_dt))
            fold_weight(wvT_sb, 2)

            # vT[pix, c_out] = x^T wv'T  (+bias_v' folded into attn output later)
            vT_sb = vT_pool.tile([P, N // P, C], dt.bfloat16)
            for pc in range(N // P):
                ps = ps_conv.tile([P, C], dt.float32, tag="conv", bufs=4)
                for cc in range(CC):
                    nc.tensor.matmul(ps[:], x_cols(cc, pc * P, P), wvT_sb[:, cc, :],
                                     start=(cc == 0), stop=(cc == CC - 1))
                nc.scalar.copy(vT_sb[:, pc, :], ps[:])
            wv_stack.close()

            wq_stack = ExitStack()
            wq_pool = wq_stack.enter_context(tc.tile_pool(name="wq_pool", bufs=1, side="right"))
            wqT_sb = wq_pool.tile([P, CC, C], qk_dt)
            for cc in range(CC):
                nc.sync.dma_start(out=wqT_sb[:, cc, :],
                                  in_=wqT_ext[cc * P:(cc + 1) * P, :].bitcast(qk_dt))
            fold_weight(wqT_sb, 0)

            q_t = [q_pool.tile([P, NQ], qk_dt, name=f"q_{oc}", tag=f"q_{oc}") for oc in range(CC)]
            for oc in range(CC):
                for ncol in range(NQ // 512):
                    ps = ps_conv.tile([P, 512], dt.float32, tag="conv", bufs=4)
                    for cc in range(CC):
                        nc.tensor.matmul(ps[:], wqT_sb[:, cc, oc * P:(oc + 1) * P],
                                         xq_t[cc][:, ncol * 512:(ncol + 1) * 512],
                                         start=(cc == 0), stop=(cc == CC - 1))
                    if ncol % 2 == 0:
                        nc.vector.tensor_scalar(
                            out=q_t[oc][:, ncol * 512:(ncol + 1) * 512], in0=ps[:],
                            scalar1=bx_sb[:, oc, 0:1], scalar2=None,
                            op0=mybir.AluOpType.add)
                    else:
                        nc.scalar.activation(
                            out=q_t[oc][:, ncol * 512:(ncol + 1) * 512], in_=ps[:],
                            func=mybir.ActivationFunctionType.Identity,
                            bias=bx_sb[:, oc, 0:1], scale=1.0)
            wq_stack.close()
            conv_ps_stack.close()
            af_stack.close()
            ho_stack.close()
            hq_stack.close()

            # ---------------- Phase 3: attention ----------------
            at_stack = ExitStack()
            at = at_stack.enter_context(tc.tile_pool(name="at", bufs=2, side="left"))
            wT_pool = at_stack.enter_context(tc.tile_pool(name="wT_pool", bufs=1, side="left"))
            out_pool = at_stack.enter_context(tc.tile_pool(name="out_pool", bufs=2, side="left"))
            ps_sc = at_stack.enter_context(
                tc.tile_pool(name="ps_sc", bufs=2, space="PSUM", side="left"))
            ps_tp = at_stack.enter_context(
                tc.tile_pool(name="ps_tp", bufs=2, space="PSUM", side="left"))
            ps_at = at_stack.enter_context(
                tc.tile_pool(name="ps_at", bufs=2, space="PSUM", side="left"))

            ident = at.tile([P, P], dt.bfloat16, tag="ident", bufs=1)
            make_identity(nc, ident[:])
            woT_sb = at.tile([P, CC, C], dt.bfloat16, tag="woT", bufs=1)
            nc.gpsimd.dma_start(out=woT_sb[:], in_=woT_ext.rearrange("(c p) o -> p c o", p=P))

            for qg in range(QCH // 4):  # groups of 4 query chunks (512 queries)
                wT_sb = wT_pool.tile([P, N // P, 512], dt.bfloat16, tag="wT")
                for qi4 in range(4):
                    qi = qg * 4 + qi4
                    # --- scores + online softmax over 4 quarters of k ---
                    e_q = at.tile([P, NQW, NKQ], dt.bfloat16, tag="e", bufs=2)
                    mq = at.tile([P, NQW], dt.float32, tag="mq")
                    sq = at.tile([P, NQW], dt.float32, tag="sq")
                    bias_t = at.tile([P, NQW], dt.float32, tag="bias")
                    for w in range(NQW):
                        ps = ps_sc.tile([P, NKQ], dt.float32, tag="sc", bufs=2)
                        for half in range(2):
                            col0 = w * NKQ + half * 512
                            for cc in range(CC):
                                nc.tensor.matmul(
                                    ps[:, half * 512:(half + 1) * 512],
                                    q_t[cc][:, qi * P:(qi + 1) * P],
                                    k_t[cc][:, col0:col0 + 512],
                                    start=(cc == 0), stop=(cc == CC - 1))
                        nc.vector.reduce_max(out=mq[:, w:w + 1], in_=ps[:],
                                             axis=mybir.AxisListType.X)
                        nc.vector.tensor_scalar_mul(bias_t[:, w:w + 1], mq[:, w:w + 1],
                                                    -SCALE)
                        nc.scalar.activation(
                            out=e_q[:, w, :], in_=ps[:],
                            func=mybir.ActivationFunctionType.Exp,
                            bias=bias_t[:, w:w + 1], scale=SCALE,
                            accum_out=sq[:, w:w + 1])
                    # combine quarters: m = max_w mq ; alpha_w = exp(SCALE*(mq-m))/s
                    m_t = at.tile([P, 1], dt.float32, tag="m")
                    nc.vector.reduce_max(out=m_t[:], in_=mq[:], axis=mybir.AxisListType.X)
                    mb = at.tile([P, 1], dt.float32, tag="mb")
                    nc.vector.tensor_scalar_mul(mb[:], m_t[:], -SCALE)
                    beta = at.tile([P, NQW], dt.float32, tag="beta")
                    nc.scalar.activation(out=beta[:], in_=mq[:],
                                         func=mybir.ActivationFunctionType.Exp,
                                         bias=mb[:], scale=SCALE)
                    sb_t = at.tile([P, NQW], dt.float32, tag="sbt")
                    nc.vector.tensor_mul(sb_t[:], sq[:], beta[:])
                    s_t = at.tile([P, 1], dt.float32, tag="s")
                    nc.vector.reduce_sum(out=s_t[:], in_=sb_t[:], axis=mybir.AxisListType.X)
                    rs = at.tile([P, 1], dt.float32, tag="rs")
                    nc.vector.reciprocal(rs[:], s_t[:])
                    alpha = at.tile([P, NQW], dt.float32, tag="alpha")
                    nc.vector.tensor_scalar_mul(alpha[:], beta[:], rs[:])
                    # normalize e, then transpose into wT columns for this chunk
                    for w in range(NQW):
                        nc.vector.tensor_scalar_mul(e_q[:, w, :], e_q[:, w, :],
                                                    alpha[:, w:w + 1])
                    for w in range(NQW):
                        tp = ps_tp.tile([P, 8, P], dt.bfloat16, tag="tp", bufs=2)
                        for j in range(8):
                            nc.tensor.transpose(
                                tp[:, j, :], e_q[:, w, j * P:(j + 1) * P], ident[:])
                        dst = wT_sb[:, w * 8:w * 8 + 8, qi4 * P:(qi4 + 1) * P]
                        if w % 2 == 0:
                            nc.scalar.copy(dst, tp[:])
                        else:
                            nc.vector.tensor_copy(dst, tp[:])

                # --- attn = v @ weights^T for this 512-query group ---
                # kc-outer in oc-pairs: each wT[kc] slice is fully consumed early,
                # letting the next group's transposes start before this group ends.
                attn_sb = at.tile([P, CC, 512], dt.bfloat16, tag="attn")
                for oc0 in (0, 2):
                    ps_pair = [ps_at.tile([P, 512], dt.float32, tag="at", bufs=2,
                                          name=f"at_ps_{oc0}_{j}")
                               for j in range(2)]
                    for kc in range(N // P):
                        for j in range(2):
                            oc = oc0 + j
                            nc.tensor.matmul(ps_pair[j][:],
                                             vT_sb[:, kc, oc * P:(oc + 1) * P],
                                             wT_sb[:, kc, :],
                                             start=(kc == 0), stop=(kc == N // P - 1))
                    for j in range(2):
                        oc = oc0 + j
                        # + folded v bias (softmax weights sum to 1, so +b[c] is exact)
                        nc.scalar.activation(out=attn_sb[:, oc, :], in_=ps_pair[j][:],
                                             func=mybir.ActivationFunctionType.Identity,
                                             bias=bx_sb[:, oc, 2:3], scale=1.0)

                # --- out = wo @ attn + bo + xq ---
                for oc in range(CC):
                    ps = ps_at.tile([P, 512], dt.float32, tag="at", bufs=2)
                    for cc in range(CC):
                        nc.tensor.matmul(ps[:], woT_sb[:, cc, oc * P:(oc + 1) * P],
                                         attn_sb[:, cc, :],
                                         start=(cc == 0), stop=(cc == CC - 1))
                    xq_sb = out_pool.tile([P, 512], dt.float32, tag="xq", bufs=2)
                    nc.sync.dma_start(out=xq_sb[:],
                                      in_=xq_ext[oc * P:(oc + 1) * P, qg * 512:(qg + 1) * 512])
                    o_sb = out_pool.tile([P, 512], dt.float32, tag="o", bufs=2)
                    nc.scalar.activation(out=o_sb[:], in_=ps[:],
                                         func=mybir.ActivationFunctionType.Identity,
                                         bias=biases_sb[:, oc, 3:4], scale=1.0)
                    nc.gpsimd.tensor_add(o_sb[:], o_sb[:], xq_sb[:])
                    nc.sync.dma_start(
                        out=out_ext[oc * P:(oc + 1) * P, qg * 512:(qg + 1) * 512],
                        in_=o_sb[:])
            at_stack.close()
            top.close()

    # Force every activation onto the natural_log_exp_and_others table set so
    # the kernel never pays a mid-run ACT table swap (~2.7us each).
    import concourse.bacc as bacc_mod
    orig_tables = bacc_mod.get_activation_tables

    def one_set_tables(arch):
        t = dict(orig_tables(arch))
        return {name: (funcs if name == "natural_log_exp_and_others" else frozenset())
                for name, funcs in t.items()}

    bacc_mod.get_activation_tables = one_set_tables
    try:
        nc.compile()
    finally:
        bacc_mod.get_activation_tables = orig_tables
    return nc




def _get_nc_generic():
    if "generic" not in _CACHE:
        _CACHE["generic"] = _build_generic(GEN_QK_MODE)
    return _CACHE["generic"]


def _in_maps_generic(x, gn_weight, gn_bias, wq, bq, wk, bk, wv, bv, wo, bo):
    x = np.asarray(x, dtype=np.float32)
    f32 = lambda a: np.ascontiguousarray(np.asarray(a, dtype=np.float32))

    wqT = f32(np.asarray(wq, dtype=np.float32).T)
    wkT = f32(np.asarray(wk, dtype=np.float32).T)
    wvT = f32(np.asarray(wv, dtype=np.float32).T)
    woT = f32(np.asarray(wo, dtype=np.float32).T)
    biases = f32(np.stack([bq, bk, bv, bo], axis=1))        # [C, 4]
    gn_ab = f32(np.stack([gn_weight, gn_bias], axis=1))     # [C, 2]

    gsel = np.zeros((C, NUM_GROUPS), dtype=np.float32)
    gsel[np.arange(C), np.arange(C) // GSIZE] = 1.0 / GSIZE
    esel = np.zeros((NUM_GROUPS, C), dtype=np.float32)
    esel[np.arange(C) // GSIZE, np.arange(C)] = 1.0

    in_maps = []
    for core in range(8):
        b, half = core // 2, core % 2
        xb = x[b].reshape(C, N)
        xqb = f32(xb[:, half * NQ:(half + 1) * NQ])
        xob = f32(xb[:, (1 - half) * NQ:(2 - half) * NQ])
        in_maps.append({
            "xq": xqb, "xo": xob,
            "wqT": wqT, "wkT": wkT, "wvT": wvT, "woT": woT,
            "biases": biases, "gn_ab": gn_ab, "gsel": gsel, "esel": esel,
        })
    return in_maps




def kernel(**inputs):
    from concourse.bass_utils import run_bass_kernel_spmd

    if np.any(np.asarray(inputs["gn_bias"])) or np.any(np.asarray(inputs["bq"])):
        # generic baseline build: handles nonzero GroupNorm/q biases, which the
        # fused fast path folds away under the zero-bias assumption
        nc = _get_nc_generic()
        in_maps = _in_maps_generic(**inputs)
    else:
        nc = _get_nc_fast()
        in_maps = _in_maps_fast(**inputs)
    res = run_bass_kernel_spmd(nc, in_maps, core_ids=list(range(8)))
    return _gather(res)

